# revision 2
# baseline (speedup 1.0000x reference)
"""Swin-style window attention (nn_BasicAttentionBlock) on 8 trn2 NeuronCores.

Strategy (data-parallel over the 4096 windows, 512/core):
- Host pre-packs per-core operands in bf16:
    qT/kT : channel-major per (window, head) -> per-head 32x49 matmul operands
    vx    : [49, W*16*33] v blocks with a ones-column appended (row-sums fall
            out of the AV matmul for free)
    eb    : exp(bias^T + mask^T) combined multiplicative softmax table
- Device per window w, head h (16 heads, d=32):
    pT[m,n] = sum_d k[m,d] q[n,d]        (matmul, K=32, out 49x49 in PSUM)
    p = exp(scale * pT) * eb[w,h]        (ACT exp reading 2 PSUM banks/window,
                                          DVE bf16 multiply)
    o[n,c]  = sum_m p[m,n] * vx[m,c]     (matmul, K=49; col 32 = row sum s[n])
    out     = o[:, :32] * (1/s)          (DVE reciprocal + tensor mul,
                                          writes fp32, DMA'd out)
- Softmax skips the max-subtraction: scores*scale ~ N(0,1), exp is safe in
  fp32, and exp(-1e9 mask) underflows to exactly 0.
"""

import os
from contextlib import ExitStack

import numpy as np
import ml_dtypes

WINDOW = 7
N = 49
C = 512
H = 16
D = 32
NW = 1024
B_ = 4096
NCORES = 8
W = B_ // NCORES          # 512 windows per core
SCALE = float(D) ** -0.5

WCH = 16                  # windows per input DMA chunk
OCH = 8                   # windows per output DMA chunk

BF16 = ml_dtypes.bfloat16
# praw column-slot -> head: QK emitted j-major with bank=j%2, slot=4*(j//2)+g
HPERM = [4 * g + j for j in range(4) for g in range(4)]


def _build_nc(Wn: int):
    import concourse.bass as bass
    import concourse.tile as tile
    import concourse.bacc as bacc
    from concourse import mybir

    bf = mybir.dt.bfloat16
    f32 = mybir.dt.float32

    nc = bacc.Bacc(None, target_bir_lowering=False)
    qk = nc.dram_tensor("qk", [128, Wn * 392], bf, kind="ExternalInput")
    ve = nc.dram_tensor("ve", [49, Wn * 1312], bf, kind="ExternalInput")
    outp = nc.dram_tensor("outp", [Wn * 49, 512], f32, kind="ExternalOutput")

    nwch = (Wn + WCH - 1) // WCH
    Exp = mybir.ActivationFunctionType.Exp

    with tile.TileContext(nc) as tc, ExitStack() as ctx:
        in_pool = ctx.enter_context(tc.tile_pool(name="inp", bufs=2))
        p_pool = ctx.enter_context(tc.tile_pool(name="p", bufs=3))
        r_pool = ctx.enter_context(tc.tile_pool(name="r", bufs=3))
        o_pool = ctx.enter_context(tc.tile_pool(name="o", bufs=2))
        ps_qk = ctx.enter_context(tc.tile_pool(name="psqk", bufs=1, space="PSUM"))
        ps_av = ctx.enter_context(tc.tile_pool(name="psav", bufs=2, space="PSUM"))

        for ci in range(nwch):
            wlo = ci * WCH
            nwin = min(WCH, Wn - wlo)
            qk_sb = in_pool.tile([128, nwin * 392], bf, tag="qk")
            nc.sync.dma_start(qk_sb[:], qk[:, wlo * 392:(wlo + nwin) * 392])
            ve_sb = in_pool.tile([49, nwin * 1312], bf, tag="ve")
            nc.sync.dma_start(ve_sb[:], ve[:, wlo * 1312:(wlo + nwin) * 1312])

            for wi in range(nwin):
                w = wlo + wi
                # ---- QK^T: 16 per-head matmuls into one 2-bank PSUM tile
                pq = ps_qk.tile([49, 2048], f32, tag="pq")
                for j in range(4):
                    for g in range(4):
                        colq = wi * 392 + g * 49
                        colk = wi * 392 + 196 + g * 49
                        qcol = 512 * j + 49 * g
                        nc.tensor.matmul(
                            pq[:, qcol:qcol + 49],
                            qk_sb[32 * j:32 * j + 32, colk:colk + 49],
                            qk_sb[32 * j:32 * j + 32, colq:colq + 49],
                            start=True, stop=True,
                            tile_position=(32 * j, 0),
                        )
                # ---- softmax numerator: exp over both banks in one ACT op,
                # then multiply by exp(bias+mask) table (bf16, DVE 2x mode)
                praw = p_pool.tile([49, 784], bf, tag="praw")
                pq_b = pq[:]
                pq_ap = bass.AP(pq_b.tensor, pq_b.offset,
                                [pq_b.ap[0], [512, 4], [1, 196]])
                nc.scalar.activation(praw[:].rearrange("p (b f) -> p b f", b=4),
                                     pq_ap, Exp, scale=SCALE)
                pmul = p_pool.tile([49, 784], bf, tag="pmul")
                ebo = wi * 1312 + 528
                nc.vector.tensor_mul(pmul[:], praw[:],
                                     ve_sb[:, ebo:ebo + 784])
                # ---- AV: 16 per-head matmuls; vx has a ones column so
                # out[:, 33h+32] = row sum
                av = ps_av.tile([49, 1024], f32, tag="av")
                for s in range(16):
                    h = HPERM[s]
                    acol = 33 * h if h < 8 else 512 + 33 * (h - 8)
                    nc.tensor.matmul(
                        av[:, acol:acol + 33],
                        pmul[:, 49 * s:49 * s + 49],
                        ve_sb[:, wi * 1312 + h * 33:wi * 1312 + h * 33 + 33],
                        start=True, stop=True,
                    )
                # ---- normalize: recip of sums, multiply, write fp32
                av_ap = av[:]
                rec = r_pool.tile([49, 16], f32, tag="rec")
                sums0 = bass.AP(av_ap.tensor, av_ap.offset + 32, [av_ap.ap[0], [33, 8]])
                sums1 = bass.AP(av_ap.tensor, av_ap.offset + 512 + 32, [av_ap.ap[0], [33, 8]])
                nc.vector.reciprocal(rec[:, 0:8], sums0)
                nc.vector.reciprocal(rec[:, 8:16], sums1)

                if w % OCH == 0:
                    o_sb = o_pool.tile([49, OCH * 512], f32, tag="osb")
                o_off = (w % OCH) * 512
                o_ap = o_sb[:]
                rec_ap = rec[:]
                for b in range(2):
                    out_dst = bass.AP(o_ap.tensor, o_ap.offset + o_off + 256 * b,
                                      [o_ap.ap[0], [32, 8], [1, 32]])
                    vals = bass.AP(av_ap.tensor, av_ap.offset + 512 * b,
                                   [av_ap.ap[0], [33, 8], [1, 32]])
                    rbc = bass.AP(rec_ap.tensor, rec_ap.offset + 8 * b,
                                  [rec_ap.ap[0], [1, 8], [0, 32]])
                    nc.vector.tensor_mul(out_dst, vals, rbc)

                if w % OCH == OCH - 1 or w == Wn - 1:
                    nlast = (w % OCH) + 1
                    base = (w - nlast + 1) * 49 * 512
                    dst = bass.AP(outp[:].tensor, base,
                                  [[512, 49], [49 * 512, nlast], [1, 512]])
                    src = bass.AP(o_ap.tensor, o_ap.offset,
                                  [o_ap.ap[0], [512, nlast], [1, 512]])
                    nc.sync.dma_start(dst, src)
    nc.compile()
    return nc


def _pack_inputs(q, k, v, bias_table, mask, rel_index):
    """Host-side packing into per-core bf16 operand tables."""
    Wn = W
    # bias^T and mask^T tables (pT layout: rows=m key token, cols=n query)
    bias = np.asarray(bias_table)[np.asarray(rel_index)]        # [n, m, H]
    biasT = bias.transpose(2, 1, 0).astype(np.float32)          # [H, m, n]
    maskT = np.asarray(mask).transpose(0, 2, 1).astype(np.float32)  # [NW, m, n]
    # combined multiplicative table for the 1024 unique windows
    ebu = np.exp(biasT[None] + maskT[:, None]).astype(BF16)     # [NW, H, m, n]

    qr = np.asarray(q).reshape(B_, N, 4, 4, 32)
    kr = np.asarray(k).reshape(B_, N, 4, 4, 32)
    vr = np.asarray(v).reshape(B_, N, H, D)

    in_maps = []
    for c in range(NCORES):
        sl = slice(c * Wn, (c + 1) * Wn)
        # [w,n,g,j,d] -> [j,d,w,g,n]; qk col layout per w: [q 196 | k 196]
        qkh = np.empty((128, Wn, 392), dtype=BF16)
        qkh[:, :, :196] = qr[sl].transpose(3, 4, 0, 2, 1).reshape(128, Wn, 196)
        qkh[:, :, 196:] = kr[sl].transpose(3, 4, 0, 2, 1).reshape(128, Wn, 196)
        vx = np.ones((N, Wn, H, 33), dtype=BF16)
        vx[:, :, :, :32] = vr[sl].transpose(1, 0, 2, 3).astype(BF16)
        widx = (np.arange(c * Wn, (c + 1) * Wn)) % NW
        ebc = ebu[widx][:, HPERM].transpose(2, 0, 1, 3)          # [m, W, slot, n]
        veh = np.empty((N, Wn, 1312), dtype=BF16)
        veh[:, :, :528] = vx.reshape(N, Wn, 528)
        veh[:, :, 528:] = ebc.reshape(N, Wn, 784)
        in_maps.append({
            "qk": np.ascontiguousarray(qkh.reshape(128, Wn * 392)),
            "ve": np.ascontiguousarray(veh.reshape(N, Wn * 1312)),
        })
    return in_maps


_CACHE = {}


def kernel(q, k, v, bias_table, mask, rel_index):
    from concourse.bass_utils import run_bass_kernel_spmd

    in_maps = _pack_inputs(q, k, v, bias_table, mask, rel_index)
    if "nc" not in _CACHE:
        _CACHE["nc"] = _build_nc(W)
    nc = _CACHE["nc"]
    trace = bool(int(os.environ.get("KBENCH_TRACE", "0")))
    tmpdir = globals().get("TRACE_TMPDIR")
    res = run_bass_kernel_spmd(nc, in_maps, core_ids=list(range(NCORES)),
                               trace=trace, tmpdir=tmpdir)
    if trace:
        _CACHE["exec_time_ns"] = res.exec_time_ns
        _CACHE["results"] = res
    out = np.empty((B_, N, C), dtype=np.float32)
    for c in range(NCORES):
        out[c * W:(c + 1) * W] = res.results[c]["outp"].reshape(W, N, C)
    return out



# revision 17
# speedup vs baseline: 1.1548x; 1.1548x over previous
"""Swin-style window attention (nn_BasicAttentionBlock) on 8 trn2 NeuronCores.

Two-lane data-parallel layout (4096 windows, 512/core, processed in pairs):
even windows live on SBUF/PSUM partitions 0-48 ("lane 0"), odd windows on
partitions 64-112 ("lane 1").  ACT/DVE/GpSimd ops process both lanes per
instruction on [128, .] tiles.

DMA notes: a transfer's SDMA-engine split is RELATIVE to its partition range
(any 49-row transfer uses only engines 0-6), so every stream is padded/packed
to 128 rows to engage all 16 engines:
  qk   [128, W*392]  bf16 channel-major q|k per (window, chan-group)
  ve   [128, (W/2)*577] bf16: per pair [vx 528 | binary mask 49], even lane
       rows 0-48, odd lane rows 64-112, gap rows zero
  eb   [128, 784] bf16 resident exp(bias) table (slot order, rows repeated
       for both lanes) -- replaces the 40MB/core per-window exp(bias+mask)
       table of the v1 kernel; the mask factor rides the small binary stream
  outp [(W/2)*128, 512] bf16 (host strips gap rows)

Device per pair (16 heads, d=32):
  pT[m,n] = sum_d k[m,d] q[n,d]    band j -> PSUM bank j
                                   (concurrent matmuls from different bands
                                   must not share a bank at the same
                                   partitions; lanes share banks at disjoint
                                   rows, which is safe), two half-pair ACTs
                                   Exp(scale*pT) pipeline with the QK matmuls
  p = exp(.) * mask * ebias        mask mul on DVE (0-stride head bcast),
                                   ebias mul on the otherwise-idle GpSimd
  o[n,c] = sum_m p[m,n] vx[m,c]    vx ones column -> row sums at col 32
  out = o[:, :32] * (1/s)          one reciprocal + one broadcast-mul per pair

Softmax skips the max-subtraction: scores*scale ~ N(0,1), exp is safe in
fp32, and masked entries are exactly zeroed by the binary mask.
"""

import os
from contextlib import ExitStack

import numpy as np
import ml_dtypes

WINDOW = 7
N = 49
C = 512
H = 16
D = 32
NW = 1024
B_ = 4096
NCORES = 8
W = B_ // NCORES          # 512 windows per core
SCALE = float(D) ** -0.5

PCH = 8                   # window PAIRS per chunk (16 windows)
VEC = 577                 # ve cols per pair: 16*33 vx + 49 mask

BF16 = ml_dtypes.bfloat16
FP8 = ml_dtypes.float8_e4m3fn
# QK writes head h=4g+j (band j, group g) to PSUM bank j, col 49g; the two
# half-pair ACTs each gather a bank pair as [196|196], so praw slot
# t = 8*(j//2) + 4*(j%2) + g
SLOT2HEAD = [4 * (t % 4) + 2 * (t // 8) + (t % 8) // 4 for t in range(16)]
HEAD2SLOT = [SLOT2HEAD.index(h) for h in range(16)]


def _build_nc(Wn: int):
    import concourse.bass as bass
    import concourse.tile as tile
    import concourse.bacc as bacc
    from concourse import mybir

    bf = mybir.dt.bfloat16
    f8 = mybir.dt.float8e4
    f32 = mybir.dt.float32

    npairs = Wn // 2
    nc = bacc.Bacc(None, target_bir_lowering=False)
    qk = nc.dram_tensor("qk", [128, Wn * 392], bf, kind="ExternalInput")
    ve = nc.dram_tensor("ve", [128, npairs * VEC], bf, kind="ExternalInput")
    eb = nc.dram_tensor("eb", [128, 784], bf, kind="ExternalInput")
    outp = nc.dram_tensor("outp", [npairs * 128, 512], bf,
                          kind="ExternalOutput")

    nch = npairs // PCH
    Exp = mybir.ActivationFunctionType.Exp

    # Persistent tensors (manual slot rotation) so PSUM lane-gap partitions
    # can be zeroed once and stay valid for the whole kernel.
    pq_buf = nc.alloc_psum_tensor("pqbuf", [128, 2048], f32)
    av_buf = nc.alloc_psum_tensor("avbuf", [128, 2048], f32)
    ve_buf = nc.alloc_sbuf_tensor("vebuf", [128, 2 * PCH * VEC], bf)
    eb_buf = nc.alloc_sbuf_tensor("ebbuf", [128, 784], bf)

    with tile.TileContext(nc) as tc, ExitStack() as ctx:
        in_pool = ctx.enter_context(tc.tile_pool(name="inp", bufs=2))
        p_pool = ctx.enter_context(tc.tile_pool(name="p", bufs=3))
        r_pool = ctx.enter_context(tc.tile_pool(name="r", bufs=3))
        o_pool = ctx.enter_context(tc.tile_pool(name="o", bufs=2))

        nc.vector.memset(pq_buf[32:64, :], 0.0)
        nc.vector.memset(pq_buf[96:128, :], 0.0)
        nc.vector.memset(av_buf[32:64, :], 0.0)
        nc.vector.memset(av_buf[96:128, :], 0.0)
        nc.sync.dma_start(eb_buf[:], eb[:])

        for ci in range(nch):
            plo = ci * PCH
            qk_sb = in_pool.tile([128, PCH * 2 * 392], bf, tag="qk")
            nc.sync.dma_start(qk_sb[:], qk[:, plo * 2 * 392:(plo + PCH) * 2 * 392])
            vbase = (ci % 2) * PCH * VEC
            nc.sync.dma_start(ve_buf[:, vbase:vbase + PCH * VEC],
                              ve[:, plo * VEC:(plo + PCH) * VEC])
            o_sb = o_pool.tile([128, PCH * 512], bf, tag="osb")

            for pp in range(PCH):
                # ---- QK^T: 2 half-pairs of (2 bands x 4 groups x 2 lanes)
                praw = p_pool.tile([128, 784], bf, tag="praw")
                pm1 = p_pool.tile([128, 784], bf, tag="pm1")
                pmul = p_pool.tile([128, 784], bf, tag="pmul")
                mcol = vbase + pp * VEC + 528
                for half in range(2):
                    for s in range(16):
                        L = s & 1
                        jj = (s >> 1) // 4
                        g = (s >> 1) % 4
                        j = 2 * half + jj
                        pbase = 64 * L
                        wi = pp * 2 + L
                        colq = wi * 392 + 49 * g
                        colk = wi * 392 + 196 + 49 * g
                        ocol = 512 * j + 49 * g
                        nc.tensor.matmul(
                            pq_buf[pbase:pbase + 49, ocol:ocol + 49],
                            qk_sb[32 * j:32 * j + 32, colk:colk + 49],
                            qk_sb[32 * j:32 * j + 32, colq:colq + 49],
                            start=True, stop=True,
                            tile_position=(32 * j, pbase),
                        )
                    hcol = 392 * half
                    # softmax numerator for this half (both lanes, one ACT op)
                    pq_b = pq_buf[:, 1024 * half:1024 * half + 1024]
                    pq_ap = bass.AP(pq_b.tensor, pq_b.offset,
                                    [pq_b.ap[0], [512, 2], [1, 196]])
                    pr_b = praw[:, hcol:hcol + 392]
                    nc.scalar.activation(
                        pr_b.rearrange("p (b f) -> p b f", b=2),
                        pq_ap, Exp, scale=SCALE)
                    # x binary mask (DVE, head-broadcast via 0-stride) then
                    # x resident exp(bias) table (GpSimd)
                    ve_b = ve_buf[:, mcol:mcol + 49]
                    mask_ap = bass.AP(ve_b.tensor, ve_b.offset,
                                      [ve_b.ap[0], [0, 8], [1, 49]])
                    pm1_b = pm1[:, hcol:hcol + 392]
                    nc.vector.tensor_mul(
                        pm1_b.rearrange("p (b f) -> p b f", b=8),
                        praw[:, hcol:hcol + 392].rearrange(
                            "p (b f) -> p b f", b=8),
                        mask_ap)
                    nc.gpsimd.tensor_mul(
                        pmul[:, hcol:hcol + 392],
                        pm1[:, hcol:hcol + 392],
                        eb_buf[:, hcol:hcol + 392])
                # ---- AV: 32 per-head matmuls; vx ones column -> row sums
                avcol = ((ci * PCH + pp) % 2) * 1024
                for s in range(32):
                    L = s & 1
                    h = s >> 1
                    pbase = 64 * L
                    acol = avcol + (33 * h if h < 8 else 512 + 33 * (h - 8))
                    vcol = vbase + pp * VEC + 33 * h
                    tcol = 49 * HEAD2SLOT[h]
                    nc.tensor.matmul(
                        av_buf[pbase:pbase + 49, acol:acol + 33],
                        pmul[pbase:pbase + 49, tcol:tcol + 49],
                        ve_buf[pbase:pbase + 49, vcol:vcol + 33],
                        start=True, stop=True,
                        tile_position=(pbase, pbase),
                    )
                # ---- normalize: one reciprocal + one broadcast-mul per pair
                av_ap = av_buf[:, avcol:avcol + 1024]
                rec = r_pool.tile([128, 16], f32, tag="rec")
                sums = bass.AP(av_ap.tensor, av_ap.offset + 32,
                               [av_ap.ap[0], [512, 2], [33, 8]])
                nc.vector.reciprocal(rec[:].rearrange("p (b f) -> p b f", b=2),
                                     sums)
                o_ap = o_sb[:]
                rec_ap = rec[:]
                out_dst = bass.AP(o_ap.tensor, o_ap.offset + pp * 512,
                                  [o_ap.ap[0], [256, 2], [32, 8], [1, 32]])
                vals = bass.AP(av_ap.tensor, av_ap.offset,
                               [av_ap.ap[0], [512, 2], [33, 8], [1, 32]])
                rbc = bass.AP(rec_ap.tensor, rec_ap.offset,
                              [rec_ap.ap[0], [8, 2], [1, 8], [0, 32]])
                nc.vector.tensor_mul(out_dst, vals, rbc)

            # ---- one full-width output DMA for the whole chunk
            src = o_sb[:]
            dst = bass.AP(outp[:].tensor, plo * 128 * 512,
                          [[512, 128], [128 * 512, PCH], [1, 512]])
            nc.sync.dma_start(dst, src)
    nc.compile()
    return nc


def _pack_inputs(q, k, v, bias_table, mask, rel_index):
    """Host-side packing into per-core operand tables."""
    # bias^T table in slot order (pT layout: rows=m key token, cols=n query)
    bias = np.asarray(bias_table)[np.asarray(rel_index)]        # [n, m, H]
    biasT = bias.transpose(2, 1, 0).astype(np.float32)          # [H, m, n]
    ebias = np.exp(biasT)[SLOT2HEAD].transpose(1, 0, 2)         # [m, slot, n]
    ebt = np.zeros((128, 784), dtype=BF16)
    ebt[0:49] = ebias.reshape(49, 784).astype(BF16)
    ebt[64:113] = ebt[0:49]
    # binary keep-mask, transposed to [m, n]
    bmask = (np.asarray(mask) > -1.0).astype(BF16)              # [NW, n, m]
    bmaskT = bmask.transpose(0, 2, 1)                           # [NW, m, n]

    qr = np.asarray(q).reshape(B_, N, 4, 4, 32)
    kr = np.asarray(k).reshape(B_, N, 4, 4, 32)
    vr = np.asarray(v).reshape(B_, N, H, D)

    npairs = W // 2
    in_maps = []
    for c in range(NCORES):
        sl = slice(c * W, (c + 1) * W)
        # [w,n,g,j,d] -> [j,d,w,g,n]; qk col layout per w: [q 196 | k 196]
        qkh = np.empty((128, W, 392), dtype=BF16)
        qkh[:, :, :196] = qr[sl].transpose(3, 4, 0, 2, 1).reshape(128, W, 196)
        qkh[:, :, 196:] = kr[sl].transpose(3, 4, 0, 2, 1).reshape(128, W, 196)
        vx = np.ones((N, W, H, 33), dtype=BF16)
        vx[:, :, :, :32] = vr[sl].transpose(1, 0, 2, 3).astype(BF16)
        vxw = vx.reshape(N, W, 528)
        widx = (np.arange(c * W, (c + 1) * W)) % NW
        mk = bmaskT[widx].transpose(1, 0, 2).astype(BF16)        # [m, W, n]
        veh = np.zeros((128, npairs, VEC), dtype=BF16)
        veh[0:49, :, 0:528] = vxw[:, 0::2]
        veh[64:113, :, 0:528] = vxw[:, 1::2]
        veh[0:49, :, 528:577] = mk[:, 0::2]
        veh[64:113, :, 528:577] = mk[:, 1::2]
        in_maps.append({
            "qk": np.ascontiguousarray(qkh.reshape(128, W * 392)),
            "ve": np.ascontiguousarray(veh.reshape(128, npairs * VEC)),
            "eb": ebt,
        })
    return in_maps


_CACHE = {}


def kernel(q, k, v, bias_table, mask, rel_index):
    from concourse.bass_utils import run_bass_kernel_spmd

    in_maps = _pack_inputs(q, k, v, bias_table, mask, rel_index)
    if "nc" not in _CACHE:
        _CACHE["nc"] = _build_nc(W)
    nc = _CACHE["nc"]
    trace = bool(int(os.environ.get("KBENCH_TRACE", "0")))
    tmpdir = globals().get("TRACE_TMPDIR")
    res = run_bass_kernel_spmd(nc, in_maps, core_ids=list(range(NCORES)),
                               trace=trace, tmpdir=tmpdir)
    if trace:
        _CACHE["exec_time_ns"] = res.exec_time_ns
        _CACHE["results"] = res
    out = np.empty((B_, N, C), dtype=np.float32)
    npairs = W // 2
    for c in range(NCORES):
        r = np.asarray(res.results[c]["outp"], dtype=np.float32)
        r = r.reshape(npairs, 128, C)
        out[c * W + 0:(c + 1) * W:2] = r[:, 0:49]
        out[c * W + 1:(c + 1) * W:2] = r[:, 64:113]
    return out


# revision 18
# speedup vs baseline: 1.4052x; 1.2169x over previous
"""Swin-style window attention (nn_BasicAttentionBlock) on 8 trn2 NeuronCores.

Two-lane data-parallel layout (4096 windows, 512/core, processed in pairs):
even windows live on SBUF/PSUM partitions 0-48 ("lane 0"), odd windows on
partitions 64-112 ("lane 1").  ACT/DVE ops process both lanes per
instruction on [128, .] tiles.

DMA notes: a transfer's SDMA-engine split is RELATIVE to its partition range
(any 49-row transfer uses only engines 0-6), so every stream is padded/packed
to 128 rows to engage all 16 engines:
  qk   [128, W*392]  bf16 channel-major q|k per (window, chan-group)
  ve   [128, (W/2)*528] bf16 v blocks + ones column (row sums ride the AV
       matmul); even lane rows 0-48, odd rows 64-112, gap rows zero
  ebs  [128, (W/2)*784] fp8e3 combined exp(bias^T + mask^T) multiplicative
       softmax table (e3m4 is plenty for values in [0, 1.2])
  outp [(W/2)*128, 512] bf16 (host strips gap rows)

Device per pair (16 heads, d=32):
  pT[m,n] = sum_d k[m,d] q[n,d]    band j -> PSUM bank j (concurrent matmuls
                                   from different bands must not share a bank
                                   at the same partitions; lanes share banks
                                   at disjoint rows, which is safe); the two
                                   half-pair ACTs Exp(scale*pT) pipeline with
                                   the QK matmuls
  p = exp(.) * ebs                 one DVE multiply per half (bf16 x fp8e3)
  o[n,c] = sum_m p[m,n] vx[m,c]    vx ones column -> row sums at col 32
  out = o[:, :32] * (1/s)          one reciprocal + one broadcast-mul per pair

Softmax skips the max-subtraction: scores*scale ~ N(0,1), exp is safe in
fp32, and exp(-1e9 mask) underflows to exactly 0 in the table.
"""

import os
from contextlib import ExitStack

import numpy as np
import ml_dtypes

WINDOW = 7
N = 49
C = 512
H = 16
D = 32
NW = 1024
B_ = 4096
NCORES = 8
W = B_ // NCORES          # 512 windows per core
SCALE = float(D) ** -0.5

PCH = 8                   # window PAIRS per chunk (16 windows)

BF16 = ml_dtypes.bfloat16
F8E3 = ml_dtypes.float8_e3m4
# QK writes head h=4g+j (band j, group g) to PSUM bank j, col 49g; the two
# half-pair ACTs each gather a bank pair as [196|196], so praw slot
# t = 8*(j//2) + 4*(j%2) + g
SLOT2HEAD = [4 * (t % 4) + 2 * (t // 8) + (t % 8) // 4 for t in range(16)]
HEAD2SLOT = [SLOT2HEAD.index(h) for h in range(16)]


def _build_nc(Wn: int):
    import concourse.bass as bass
    import concourse.tile as tile
    import concourse.bacc as bacc
    from concourse import mybir

    bf = mybir.dt.bfloat16
    f8 = mybir.dt.float8e3
    f32 = mybir.dt.float32

    npairs = Wn // 2
    nc = bacc.Bacc(None, target_bir_lowering=False)
    qk = nc.dram_tensor("qk", [128, Wn * 392], bf, kind="ExternalInput")
    ve = nc.dram_tensor("ve", [128, npairs * 528], bf, kind="ExternalInput")
    ebs = nc.dram_tensor("ebs", [128, npairs * 784], f8, kind="ExternalInput")
    outp = nc.dram_tensor("outp", [npairs * 128, 512], bf,
                          kind="ExternalOutput")

    nch = npairs // PCH
    Exp = mybir.ActivationFunctionType.Exp

    # Persistent PSUM tensors so the lane-gap partitions (49-63, 113-127) can
    # be zeroed once and stay valid for the whole kernel.
    pq_buf = nc.alloc_psum_tensor("pqbuf", [128, 2048], f32)
    av_buf = nc.alloc_psum_tensor("avbuf", [128, 2048], f32)

    with tile.TileContext(nc) as tc, ExitStack() as ctx:
        in_pool = ctx.enter_context(tc.tile_pool(name="inp", bufs=2))
        p_pool = ctx.enter_context(tc.tile_pool(name="p", bufs=3))
        r_pool = ctx.enter_context(tc.tile_pool(name="r", bufs=3))
        o_pool = ctx.enter_context(tc.tile_pool(name="o", bufs=2))

        nc.vector.memset(pq_buf[32:64, :], 0.0)
        nc.vector.memset(pq_buf[96:128, :], 0.0)
        nc.vector.memset(av_buf[32:64, :], 0.0)
        nc.vector.memset(av_buf[96:128, :], 0.0)

        for ci in range(nch):
            plo = ci * PCH
            qk_sb = in_pool.tile([128, PCH * 2 * 392], bf, tag="qk")
            nc.sync.dma_start(qk_sb[:], qk[:, plo * 2 * 392:(plo + PCH) * 2 * 392])
            ve_sb = in_pool.tile([128, PCH * 528], bf, tag="ve")
            nc.sync.dma_start(ve_sb[:], ve[:, plo * 528:(plo + PCH) * 528])
            eb_sb = in_pool.tile([128, PCH * 784], f8, tag="ebs")
            nc.sync.dma_start(eb_sb[:], ebs[:, plo * 784:(plo + PCH) * 784])
            o_sb = o_pool.tile([128, PCH * 512], bf, tag="osb")

            for pp in range(PCH):
                # ---- QK^T: 2 half-pairs of (2 bands x 4 groups x 2 lanes)
                praw = p_pool.tile([128, 784], bf, tag="praw")
                pmul = p_pool.tile([128, 784], bf, tag="pmul")
                for half in range(2):
                    for s in range(16):
                        L = s & 1
                        jj = (s >> 1) // 4
                        g = (s >> 1) % 4
                        j = 2 * half + jj
                        pbase = 64 * L
                        wi = pp * 2 + L
                        colq = wi * 392 + 49 * g
                        colk = wi * 392 + 196 + 49 * g
                        ocol = 512 * j + 49 * g
                        nc.tensor.matmul(
                            pq_buf[pbase:pbase + 49, ocol:ocol + 49],
                            qk_sb[32 * j:32 * j + 32, colk:colk + 49],
                            qk_sb[32 * j:32 * j + 32, colq:colq + 49],
                            start=True, stop=True,
                            tile_position=(32 * j, pbase),
                        )
                    hcol = 392 * half
                    # softmax numerator for this half (both lanes, one ACT op)
                    pq_b = pq_buf[:, 1024 * half:1024 * half + 1024]
                    pq_ap = bass.AP(pq_b.tensor, pq_b.offset,
                                    [pq_b.ap[0], [512, 2], [1, 196]])
                    pr_b = praw[:, hcol:hcol + 392]
                    nc.scalar.activation(
                        pr_b.rearrange("p (b f) -> p b f", b=2),
                        pq_ap, Exp, scale=SCALE)
                    # x combined exp(bias+mask) table (bf16 x fp8e3 on DVE)
                    ecol = pp * 784 + hcol
                    nc.vector.tensor_mul(
                        pmul[:, hcol:hcol + 392],
                        praw[:, hcol:hcol + 392],
                        eb_sb[:, ecol:ecol + 392])
                # ---- AV: 32 per-head matmuls; vx ones column -> row sums
                avcol = ((ci * PCH + pp) % 2) * 1024
                for s in range(32):
                    L = s & 1
                    h = s >> 1
                    pbase = 64 * L
                    acol = avcol + (33 * h if h < 8 else 512 + 33 * (h - 8))
                    vcol = pp * 528 + 33 * h
                    tcol = 49 * HEAD2SLOT[h]
                    nc.tensor.matmul(
                        av_buf[pbase:pbase + 49, acol:acol + 33],
                        pmul[pbase:pbase + 49, tcol:tcol + 49],
                        ve_sb[pbase:pbase + 49, vcol:vcol + 33],
                        start=True, stop=True,
                        tile_position=(pbase, pbase),
                    )
                # ---- normalize: one reciprocal + one broadcast-mul per pair
                av_ap = av_buf[:, avcol:avcol + 1024]
                rec = r_pool.tile([128, 16], f32, tag="rec")
                sums = bass.AP(av_ap.tensor, av_ap.offset + 32,
                               [av_ap.ap[0], [512, 2], [33, 8]])
                nc.vector.reciprocal(rec[:].rearrange("p (b f) -> p b f", b=2),
                                     sums)
                o_ap = o_sb[:]
                rec_ap = rec[:]
                out_dst = bass.AP(o_ap.tensor, o_ap.offset + pp * 512,
                                  [o_ap.ap[0], [256, 2], [32, 8], [1, 32]])
                vals = bass.AP(av_ap.tensor, av_ap.offset,
                               [av_ap.ap[0], [512, 2], [33, 8], [1, 32]])
                rbc = bass.AP(rec_ap.tensor, rec_ap.offset,
                              [rec_ap.ap[0], [8, 2], [1, 8], [0, 32]])
                nc.vector.tensor_mul(out_dst, vals, rbc)

            # ---- one full-width output DMA for the whole chunk
            src = o_sb[:]
            dst = bass.AP(outp[:].tensor, plo * 128 * 512,
                          [[512, 128], [128 * 512, PCH], [1, 512]])
            nc.sync.dma_start(dst, src)
    nc.compile()
    return nc


def _pack_inputs(q, k, v, bias_table, mask, rel_index):
    """Host-side packing into per-core operand tables."""
    # combined exp(bias^T + mask^T) table in slot order (pT layout:
    # rows=m key token, cols=n query)
    bias = np.asarray(bias_table)[np.asarray(rel_index)]        # [n, m, H]
    biasT = bias.transpose(2, 1, 0).astype(np.float32)          # [H, m, n]
    maskT = np.asarray(mask).transpose(0, 2, 1).astype(np.float32)  # [NW, m, n]
    ebu = np.exp(biasT[None] + maskT[:, None])                  # [NW, H, m, n]
    ebu = ebu[:, SLOT2HEAD].astype(F8E3)                        # [NW, slot, m, n]

    qr = np.asarray(q).reshape(B_, N, 4, 4, 32)
    kr = np.asarray(k).reshape(B_, N, 4, 4, 32)
    vr = np.asarray(v).reshape(B_, N, H, D)

    npairs = W // 2
    in_maps = []
    for c in range(NCORES):
        sl = slice(c * W, (c + 1) * W)
        # [w,n,g,j,d] -> [j,d,w,g,n]; qk col layout per w: [q 196 | k 196]
        qkh = np.empty((128, W, 392), dtype=BF16)
        qkh[:, :, :196] = qr[sl].transpose(3, 4, 0, 2, 1).reshape(128, W, 196)
        qkh[:, :, 196:] = kr[sl].transpose(3, 4, 0, 2, 1).reshape(128, W, 196)
        vx = np.ones((N, W, H, 33), dtype=BF16)
        vx[:, :, :, :32] = vr[sl].transpose(1, 0, 2, 3).astype(BF16)
        vxw = vx.reshape(N, W, 528)
        veh = np.zeros((128, npairs, 528), dtype=BF16)
        veh[0:49] = vxw[:, 0::2]
        veh[64:113] = vxw[:, 1::2]
        widx = (np.arange(c * W, (c + 1) * W)) % NW
        ebc = ebu[widx].transpose(2, 0, 1, 3).reshape(N, W, 784)  # [m, W, 784]
        ebh = np.zeros((128, npairs, 784), dtype=F8E3)
        ebh[0:49] = ebc[:, 0::2]
        ebh[64:113] = ebc[:, 1::2]
        in_maps.append({
            "qk": np.ascontiguousarray(qkh.reshape(128, W * 392)),
            "ve": np.ascontiguousarray(veh.reshape(128, npairs * 528)),
            "ebs": np.ascontiguousarray(ebh.reshape(128, npairs * 784)),
        })
    return in_maps


_CACHE = {}


def kernel(q, k, v, bias_table, mask, rel_index):
    from concourse.bass_utils import run_bass_kernel_spmd

    in_maps = _pack_inputs(q, k, v, bias_table, mask, rel_index)
    if "nc" not in _CACHE:
        _CACHE["nc"] = _build_nc(W)
    nc = _CACHE["nc"]
    trace = bool(int(os.environ.get("KBENCH_TRACE", "0")))
    tmpdir = globals().get("TRACE_TMPDIR")
    res = run_bass_kernel_spmd(nc, in_maps, core_ids=list(range(NCORES)),
                               trace=trace, tmpdir=tmpdir)
    if trace:
        _CACHE["exec_time_ns"] = res.exec_time_ns
        _CACHE["results"] = res
    out = np.empty((B_, N, C), dtype=np.float32)
    npairs = W // 2
    for c in range(NCORES):
        r = np.asarray(res.results[c]["outp"], dtype=np.float32)
        r = r.reshape(npairs, 128, C)
        out[c * W + 0:(c + 1) * W:2] = r[:, 0:49]
        out[c * W + 1:(c + 1) * W:2] = r[:, 64:113]
    return out
